# revision 1
# baseline (speedup 1.0000x reference)
"""DiagonalBandAttention Trainium2 kernel.

Computation (reference semantics):
  band[b,c,j]  = mean_{k=0..20} xpad[b,c,j+k,j]        (rows zero-padded by 10)
  conv[b,c,s]  = depthwise_conv1d(band, conv_w, k=7, pad=3)   (cross-correlation)
  attn[b,d,s]  = softmax_s( sum_c point_w[d,c]*conv[b,c,s] + point_b[d] )
  out          = x, with out[b,c,j,j] = x[b,c,j,j] * attn[b,c,j]

Output is x copied verbatim except the main diagonal of each [S,S] map.
The kernel is memory-bound on the x -> out copy (2 * 384 MB).

Sharding (8 cores): core k handles batch b = k//4, channels [48*(k%4), 48*(k%4)+48).
Each core:
  - bulk-copies its x shard DRAM->DRAM,
  - receives the diagonal-band slices E[b] = xpad[b,:,j+k,j] of its whole batch
    (all 192 channels are needed because the 1x1 conv mixes channels),
  - computes band-mean -> depthwise conv -> pointwise matmul -> softmax on chip,
  - scatters the rescaled diagonal into the copied output.
"""

import numpy as np

B, C, S = 2, 192, 512
BW = 21          # band width
HALF = BW // 2   # 10
K = 7            # depthwise conv taps
CSH = C // 4     # 48 channels per core
N_CORES = 8
BULK_CH = 4      # channels per bulk copy DMA

_prog = {}


def _build_program(debug=False):
    """Raw-bass program (Tile's sem assignment emits multi-wait compute
    instructions that this walrus rejects, so sync is managed manually).

    Engine plan:
      SP     - 12 big DRAM->DRAM copies x_sh -> out        (bulk sem)
      ACT    - input DMAs, exp, final diagonal scatter      (din/asem)
      DVE    - band sum, depthwise conv, softmax arithmetic (vs)
      PE     - 1x1 conv matmuls into PSUM                   (psem)

    Cross-engine deps (all single-sem standalone waits):
      DVE waits din>=128 (all 8 input DMAs)   -> band/conv -> vs=1
      PE  waits vs>=1                          -> matmuls  -> psem=1
      DVE waits psem>=1                        -> bias+negmax -> vs=3
      ACT waits vs>=3                          -> exp+sum  -> asem=1
      DVE waits asem>=1                        -> dv       -> vs=4
      ACT waits vs>=4 and bulk>=192            -> diag scatter -> din=144
    """
    import concourse.bass as bass
    import concourse.mybir as mybir

    f32 = mybir.dt.float32
    Alu = mybir.AluOpType
    N_BULK = CSH // BULK_CH

    nc = bass.Bass()
    x_sh = nc.declare_dram_parameter("x_sh", [CSH, S, S], f32, isOutput=False)
    e_b = nc.declare_dram_parameter("e_b", [C, BW, S], f32, isOutput=False)
    xdg = nc.declare_dram_parameter("xdg", [CSH, S], f32, isOutput=False)
    cw = nc.declare_dram_parameter("cw", [C, K], f32, isOutput=False)
    pwt = nc.declare_dram_parameter("pwt", [256, CSH], f32, isOutput=False)
    pb = nc.declare_dram_parameter("pb", [CSH, 1], f32, isOutput=False)
    out = nc.declare_dram_parameter("out", [CSH, S, S], f32, isOutput=True)
    dbg = {}
    if debug:
        for name, shape in (
            ("band_o", [128, S + K - 1]), ("ct_o", [128, S]), ("sm_o", [CSH, S]),
            ("ex_o", [CSH, S]), ("ssum_o", [CSH, 1]), ("rinv_o", [CSH, 1]),
            ("dv_o", [CSH, S]),
        ):
            dbg[name] = nc.declare_dram_parameter(name, shape, f32, isOutput=True)

    x_flat = x_sh.ap().rearrange("c h w -> c (h w)")
    out_flat = out.ap().rearrange("c h w -> c (h w)")
    e_ap = e_b.ap()
    cw_ap = cw.ap()
    pwt_ap = pwt.ap()

    from contextlib import ExitStack

    with ExitStack() as ctx:
        et1 = ctx.enter_context(nc.sbuf_tensor([128, BW, S], f32))
        et2 = ctx.enter_context(nc.sbuf_tensor([64, BW, S], f32))
        band1 = ctx.enter_context(nc.sbuf_tensor([128, S + K - 1], f32))
        band2 = ctx.enter_context(nc.sbuf_tensor([64, S + K - 1], f32))
        ct1 = ctx.enter_context(nc.sbuf_tensor([128, S], f32))
        ct2 = ctx.enter_context(nc.sbuf_tensor([128, S], f32))
        cw1 = ctx.enter_context(nc.sbuf_tensor([128, K], f32))
        cw2 = ctx.enter_context(nc.sbuf_tensor([64, K], f32))
        pw1 = ctx.enter_context(nc.sbuf_tensor([128, CSH], f32))
        pw2 = ctx.enter_context(nc.sbuf_tensor([128, CSH], f32))
        pbt = ctx.enter_context(nc.sbuf_tensor([CSH, 1], f32))
        sm = ctx.enter_context(nc.sbuf_tensor([CSH, S], f32))
        negmax = ctx.enter_context(nc.sbuf_tensor([CSH, 1], f32))
        ex = ctx.enter_context(nc.sbuf_tensor([CSH, S], f32))
        ssum = ctx.enter_context(nc.sbuf_tensor([CSH, 1], f32))
        rinv = ctx.enter_context(nc.sbuf_tensor([CSH, 1], f32))
        lse = ctx.enter_context(nc.sbuf_tensor([CSH, 1], f32))
        nrt = ctx.enter_context(nc.sbuf_tensor([CSH, 1], f32))
        xdgt = ctx.enter_context(nc.sbuf_tensor([CSH, S], f32))
        dv = ctx.enter_context(nc.sbuf_tensor([CSH, S], f32))
        ps = ctx.enter_context(nc.psum_tensor([CSH, S], f32))
        din = ctx.enter_context(nc.semaphore("din"))
        bulk = ctx.enter_context(nc.semaphore("bulk"))
        vs = ctx.enter_context(nc.semaphore("vs"))
        psem = ctx.enter_context(nc.semaphore("psem"))
        asem = ctx.enter_context(nc.semaphore("asem"))
        block = ctx.enter_context(nc.Block())

        @block.sync
        def _(sync):
            # inputs first: their completion starves behind bulk packets in
            # the SDMA round-robin otherwise, stalling compute ~400us
            sync.wait_ge(din, 128)
            for i in range(N_BULK):
                sync.dma_start(
                    out=out_flat[i * BULK_CH : (i + 1) * BULK_CH, :],
                    in_=x_flat[i * BULK_CH : (i + 1) * BULK_CH, :],
                ).then_inc(bulk, 16)

        @block.scalar
        def _(scalar):
            scalar.dma_start(out=et1[:], in_=e_ap[0:128]).then_inc(din, 16)
            scalar.dma_start(out=et2[:], in_=e_ap[128:C]).then_inc(din, 16)
            scalar.dma_start(out=cw1[:], in_=cw_ap[0:128]).then_inc(din, 16)
            scalar.dma_start(out=cw2[:], in_=cw_ap[128:C]).then_inc(din, 16)
            scalar.dma_start(out=pw1[:], in_=pwt_ap[0:128]).then_inc(din, 16)
            scalar.dma_start(out=pw2[:], in_=pwt_ap[128:256]).then_inc(din, 16)
            scalar.dma_start(out=pbt[:], in_=pb.ap()).then_inc(din, 16)
            scalar.dma_start(out=xdgt[:], in_=xdg.ap()).then_inc(din, 16)
            scalar.wait_ge(vs, 3)
            scalar.activation(
                out=ex[:], in_=sm[:], func=mybir.ActivationFunctionType.Exp,
                bias=negmax[:], scale=1.0,
            ).then_inc(asem, 1)
            # seed 1/ssum = exp(-ln(ssum)); DVE Newton-polishes it
            scalar.wait_ge(vs, 4)
            scalar.activation(
                out=lse[:], in_=ssum[:], func=mybir.ActivationFunctionType.Ln
            )
            scalar.activation(
                out=rinv[:], in_=lse[:], func=mybir.ActivationFunctionType.Exp,
                scale=-1.0,
            ).then_inc(asem, 1)
            scalar.wait_ge(vs, 5)
            # diagonal scatter per bulk chunk, each ordered after its
            # chunk's copy so the (slow, 4B-RMW) descriptors overlap the
            # remaining bulk instead of serializing at the end
            n_dma = 8 + N_BULK
            with nc.allow_non_contiguous_dma(reason="diagonal scatter"):
                for i in range(N_BULK):
                    scalar.wait_ge(bulk, 16 * (i + 1))
                    scalar.dma_start(
                        out=out_flat[
                            i * BULK_CH : (i + 1) * BULK_CH, 0 : S * S : S + 1
                        ],
                        in_=dv[i * BULK_CH : (i + 1) * BULK_CH, :],
                    ).then_inc(din, 16)
            if debug:
                for name, src in (
                    ("band_o", band1), ("ct_o", ct1), ("sm_o", sm), ("ex_o", ex),
                    ("ssum_o", ssum), ("rinv_o", rinv), ("dv_o", dv),
                ):
                    scalar.dma_start(out=dbg[name].ap(), in_=src[:]).then_inc(din, 16)
                    n_dma += 1
            scalar.wait_ge(din, 16 * n_dma)

        @block.vector
        def _(vector):
            vector.wait_ge(din, 128)
            # band sums over the 21 taps (mean's 1/21 folded into cw on host)
            for (band, et, p) in ((band1, et1, 128), (band2, et2, 64)):
                bs = band[0:p, 3 : 3 + S]
                vector.tensor_tensor(
                    out=bs, in0=et[0:p, 0, :], in1=et[0:p, 1, :], op=Alu.add
                )
                for k in range(2, BW):
                    vector.tensor_tensor(
                        out=bs, in0=et[0:p, k, :], in1=bs, op=Alu.add
                    )
                vector.memset(band[0:p, 0:3], 0.0)
                vector.memset(band[0:p, 3 + S :], 0.0)
            vector.memset(ct2[64:128, :], 0.0)  # zero padding partitions
            # depthwise conv, 7 taps
            for (ct, band, cwt, p) in ((ct1, band1, cw1, 128), (ct2, band2, cw2, 64)):
                vector.tensor_scalar(
                    out=ct[0:p, :], in0=band[0:p, 0:S],
                    scalar1=cwt[0:p, 0:1], scalar2=None, op0=Alu.mult,
                )
                for t in range(1, K):
                    stt = vector.scalar_tensor_tensor(
                        out=ct[0:p, :], in0=band[0:p, t : t + S],
                        scalar=cwt[0:p, t : t + 1], in1=ct[0:p, :],
                        op0=Alu.mult, op1=Alu.add,
                    )
                stt.then_inc(vs, 1)  # vs=1 after ct1, vs=2 after ct2
            vector.wait_ge(psem, 1)
            vector.tensor_scalar_add(out=sm[:], in0=ps[:], scalar1=pbt[:])
            vector.tensor_reduce(
                out=negmax[:], in_=sm[:], axis=mybir.AxisListType.X,
                op=Alu.max, negate=True,
            ).then_inc(vs, 1)  # vs=3: exp inputs ready
            vector.wait_ge(asem, 1)
            vector.tensor_reduce(
                out=ssum[:], in_=ex[:], axis=mybir.AxisListType.X, op=Alu.add
            ).then_inc(vs, 1)  # vs=4: ssum ready for ACT's 1/x seed
            vector.wait_ge(asem, 2)
            for _ in range(2):  # Newton: y <- y*(2 - x*y)
                vector.tensor_tensor(
                    out=nrt[:], in0=ssum[:], in1=rinv[:], op=Alu.mult
                )
                vector.tensor_scalar(
                    out=nrt[:], in0=nrt[:], scalar1=-1.0, scalar2=2.0,
                    op0=Alu.mult, op1=Alu.add,
                )
                vector.tensor_tensor(
                    out=rinv[:], in0=rinv[:], in1=nrt[:], op=Alu.mult
                )
            vector.tensor_tensor(out=dv[:], in0=ex[:], in1=xdgt[:], op=Alu.mult)
            vector.tensor_scalar_mul(
                out=dv[:], in0=dv[:], scalar1=rinv[:]
            ).then_inc(vs, 1)  # vs=5: dv ready

        @block.tensor
        def _(tensor):
            tensor.wait_ge(vs, 2)
            nc.tensor.matmul(ps[:], lhsT=pw1[:], rhs=ct1[:], start=True, stop=False)
            nc.tensor.matmul(
                ps[:], lhsT=pw2[:], rhs=ct2[:], start=False, stop=True
            ).then_inc(psem, 1)

    return nc


def _get_program(debug=False):
    if debug not in _prog:
        _prog[debug] = _build_program(debug)
    return _prog[debug]


def _host_prep(x, conv_w, point_w, point_b):
    """Build per-core input maps. Everything here is slicing/layout only."""
    x = np.asarray(x, dtype=np.float32)
    conv_w = np.asarray(conv_w, dtype=np.float32)
    point_w = np.asarray(point_w, dtype=np.float32)
    point_b = np.asarray(point_b, dtype=np.float32)

    # E[b,c,k,j] = xpad[b,c,j+k,j]  (rows padded by HALF), via diagonal views
    E = np.zeros((B, C, BW, S), dtype=np.float32)
    for k in range(BW):
        o = HALF - k
        d = np.diagonal(x, offset=o, axis1=2, axis2=3)
        if o >= 0:
            E[:, :, k, o:S] = d
        else:
            E[:, :, k, 0 : S + o] = d

    cw_all = np.ascontiguousarray(conv_w.reshape(C, K) / np.float32(BW))

    in_maps = []
    for core in range(N_CORES):
        b, cb = divmod(core, 4)
        c0 = cb * CSH
        pwt_sh = np.zeros((256, CSH), dtype=np.float32)
        pwt_sh[:C] = point_w[c0 : c0 + CSH, :].T
        in_maps.append(
            {
                "x_sh": np.ascontiguousarray(x[b, c0 : c0 + CSH]),
                "e_b": np.ascontiguousarray(E[b]),
                "xdg": np.ascontiguousarray(E[b, c0 : c0 + CSH, HALF, :]),
                "cw": cw_all,
                "pwt": pwt_sh,
                "pb": np.ascontiguousarray(point_b[c0 : c0 + CSH].reshape(CSH, 1)),
            }
        )
    return in_maps


def _run(inputs, trace=False, debug=False):
    from concourse.bass_utils import run_bass_kernel_spmd

    nc = _get_program(debug)
    in_maps = _host_prep(**inputs)
    res = run_bass_kernel_spmd(
        nc, in_maps, core_ids=list(range(N_CORES)), trace=trace
    )
    out = np.empty((B, C, S, S), dtype=np.float32)
    for core in range(N_CORES):
        b, cb = divmod(core, 4)
        c0 = cb * CSH
        out[b, c0 : c0 + CSH] = res.results[core]["out"]
    return out, res


def kernel(x, conv_w, point_w, point_b):
    out, _ = _run(dict(x=x, conv_w=conv_w, point_w=point_w, point_b=point_b))
    return out

